# revision 2
# baseline (speedup 1.0000x reference)
"""Trainium2 Bass kernel for nn_BackwardConv2D (CROWN linear-bound backward conv).

Math: the reference materializes the dense conv matrix Wm (n_dim x n_out) of a
3x3 SAME conv (H=W=16, CIN=COUT=8) and contracts it with w_out_u / w_out_l
per batch:  Y[b] = Wm @ Ws[b]  (the pos/neg split is linear and sums back to
the original), plus bias rows  T[b,k] = bvec . Ws[b,:,k].

Structure exploited here:
  * i = (h, w, ci) flat = 128*h + 8*w + ci ; o = (ho, wo, co) flat likewise.
    Wm[i, o] = K[h-ho+1, w-wo+1, ci, co], zero unless |h-ho|<=1 and |w-wo|<=1.
    => block-tridiagonal in 128-row tiles: output tile h only contracts with
       input tiles ho in {h-1, h, h+1}; and the 128x128 block depends only on
       dh = h-ho, so there are just 3 distinct stationary matrices.
  * 8 independent (batch, upper/lower) GEMMs -> one per NeuronCore.

Per core: banded matmul (46 nonzero 128x128x2048 tile-matmuls instead of 256
dense) + bias contraction as M=1 matmuls, all in float32r (FP22 1-pass).
"""

import numpy as np

import concourse.bass as bass  # noqa: F401  (env check)
import concourse.tile as tile
import concourse.mybir as mybir
from concourse import bacc
from concourse.bass_utils import run_bass_kernel_spmd

# Problem shape (hardcoded per harness contract)
B = 4
H = W = 16
CIN = COUT = 8
N = 2048          # n_dim = n_out = n_back = H*W*CIN
P = 128           # partition tile (one h-row of (w, ci))
NH = 16           # number of 128-row tiles (= H)
KC = 512          # k-chunk (one PSUM bank of fp32)
NCH = N // KC     # 4
N_CORES = 8

_f32 = mybir.dt.float32
_f32r = mybir.dt.float32r

_NC_CACHE = None


def _build_module():
    """One SPMD program: banded 2048^3 matmul + bias row, per core."""
    nc = bacc.Bacc("TRN2", target_bir_lowering=False, debug=False,
                   num_devices=N_CORES)

    lm_d = nc.dram_tensor("lmats", (3, P, P), _f32r, kind="ExternalInput").ap()
    bv_d = nc.dram_tensor("bv", (P, 1), _f32r, kind="ExternalInput").ap()
    ws_d = nc.dram_tensor("ws", (N, N), _f32r, kind="ExternalInput").ap()
    bo_d = nc.dram_tensor("bout", (1, N), _f32, kind="ExternalInput").ap()
    y_d = nc.dram_tensor("y", (N, N), _f32, kind="ExternalOutput").ap()
    bn_d = nc.dram_tensor("bnew", (1, N), _f32, kind="ExternalOutput").ap()

    with tile.TileContext(nc) as tc:
        with (
            tc.tile_pool(name="const", bufs=1) as cpool,
            tc.tile_pool(name="wsp", bufs=1) as wpool,
            tc.tile_pool(name="yp", bufs=4) as ypool,
            tc.tile_pool(name="bnp", bufs=2) as bnpool,
            tc.tile_pool(name="acc", bufs=6, space="PSUM") as ppool,
            tc.tile_pool(name="bacc", bufs=2, space="PSUM") as pbpool,
        ):
            lsb = cpool.tile([P, 3, P], _f32r)
            for d in range(3):
                nc.sync.dma_start(out=lsb[:, d, :], in_=lm_d[d])
            bvsb = cpool.tile([P, 1], _f32r)
            nc.sync.dma_start(out=bvsb[:], in_=bv_d[:])
            bosb = cpool.tile([1, N], _f32)
            nc.sync.dma_start(out=bosb[:], in_=bo_d[:])

            ws_sb = []
            for ho in range(NH):
                t = wpool.tile([P, N], _f32r, tag=f"ws{ho}")
                nc.sync.dma_start(out=t[:], in_=ws_d[ho * P:(ho + 1) * P, :])
                ws_sb.append(t)

            for kc in range(NCH):
                sl = slice(kc * KC, (kc + 1) * KC)
                for h in range(NH):
                    acc = ppool.tile([P, KC], _f32, tag="acc")
                    dhs = [dh for dh in (-1, 0, 1) if 0 <= h - dh < NH]
                    for j, dh in enumerate(dhs):
                        ho = h - dh
                        nc.tensor.matmul(
                            acc[:],
                            lsb[:, dh + 1, :],
                            ws_sb[ho][:, sl],
                            start=(j == 0),
                            stop=(j == len(dhs) - 1),
                        )
                    yt = ypool.tile([P, KC], _f32, tag="y")
                    nc.vector.tensor_copy(yt[:], acc[:])
                    nc.sync.dma_start(out=y_d[h * P:(h + 1) * P, sl], in_=yt[:])

                pb = pbpool.tile([1, KC], _f32, tag="bacc")
                for ho in range(NH):
                    nc.tensor.matmul(
                        pb[:],
                        bvsb[:],
                        ws_sb[ho][:, sl],
                        start=(ho == 0),
                        stop=(ho == NH - 1),
                    )
                bn = bnpool.tile([1, KC], _f32, tag="bn")
                nc.vector.tensor_add(bn[:], pb[:], bosb[:, sl])
                nc.sync.dma_start(out=bn_d[:, sl], in_=bn[:])

    nc.compile()
    return nc


def _get_nc():
    global _NC_CACHE
    if _NC_CACHE is None:
        _NC_CACHE = _build_module()
    return _NC_CACHE


def _stationaries(kernel: np.ndarray):
    """lhsT matrices: L[dh+1][(wo,co),(w,ci)] = K[dh+1, w-wo+1, ci, co]."""
    L = np.zeros((3, P, P), np.float32)
    k = np.asarray(kernel, np.float32)
    for dhi in range(3):
        for wo in range(W):
            for w in range(W):
                dw = w - wo
                if -1 <= dw <= 1:
                    L[dhi, 8 * wo:8 * wo + 8, 8 * w:8 * w + 8] = k[dhi, dw + 1].T
    return L


def run_device(inputs: dict, trace: bool = False):
    """Shard over 8 cores, run the bass kernel, gather. Returns (outs, results)."""
    nc = _get_nc()
    L = _stationaries(inputs["kernel"])
    bv = np.tile(np.asarray(inputs["bias"], np.float32), W).reshape(P, 1)

    in_maps = []
    for c in range(N_CORES):
        b, ul = c // 2, c % 2
        ws = inputs["w_out_u"] if ul == 0 else inputs["w_out_l"]
        bo = inputs["b_out_u"] if ul == 0 else inputs["b_out_l"]
        in_maps.append({
            "lmats": L,
            "bv": bv,
            "ws": np.ascontiguousarray(np.asarray(ws[b, 0], np.float32)),
            "bout": np.asarray(bo[b, 0], np.float32).reshape(1, N),
        })

    res = run_bass_kernel_spmd(nc, in_maps, core_ids=list(range(N_CORES)),
                               trace=trace)

    wu = np.stack([res.results[2 * b]["y"] for b in range(B)])[:, None]
    wl = np.stack([res.results[2 * b + 1]["y"] for b in range(B)])[:, None]
    bu = np.stack([res.results[2 * b]["bnew"][0] for b in range(B)])[:, None]
    bl = np.stack([res.results[2 * b + 1]["bnew"][0] for b in range(B)])[:, None]
    return (wu, bu, wl, bl), res


def kernel(**inputs) -> tuple:
    outs, _ = run_device(inputs, trace=False)
    return outs


# revision 6
# speedup vs baseline: 1.0090x; 1.0090x over previous
"""Trainium2 Bass kernel for nn_BackwardConv2D (CROWN linear-bound backward conv).

Math: the reference materializes the dense conv matrix Wm (n_dim x n_out) of a
3x3 SAME conv (H=W=16, CIN=COUT=8) and contracts it with w_out_u / w_out_l
per batch:  Y[b] = Wm @ Ws[b]  (the pos/neg split is linear and sums back to
the original), plus bias rows  T[b,k] = bvec . Ws[b,:,k].

Structure exploited here:
  * i = (h, w, ci) flat = 128*h + 8*w + ci ; o = (ho, wo, co) flat likewise.
    Wm[i, o] = K[h-ho+1, w-wo+1, ci, co], zero unless |h-ho|<=1 and |w-wo|<=1.
    => block-tridiagonal in 128-row tiles: output tile h only contracts with
       input tiles ho in {h-1, h, h+1}; and the 128x128 block depends only on
       dh = h-ho, so there are just 3 distinct stationary matrices.
  * 8 independent (batch, upper/lower) GEMMs -> one per NeuronCore.

Per core: banded matmul (46 nonzero 128x128x2048 tile-matmuls instead of 256
dense) + bias contraction as M=1 matmuls, all in float32r (FP22 1-pass).
"""

import numpy as np

import concourse.bass as bass  # noqa: F401  (env check)
import concourse.tile as tile
import concourse.mybir as mybir
from concourse import bacc
from concourse.bass_utils import run_bass_kernel_spmd

# Problem shape (hardcoded per harness contract)
B = 4
H = W = 16
CIN = COUT = 8
N = 2048          # n_dim = n_out = n_back = H*W*CIN
P = 128           # partition tile (one h-row of (w, ci))
NH = 16           # number of 128-row tiles (= H)
KC = 512          # k-chunk (one PSUM bank of fp32)
NCH = N // KC     # 4
N_CORES = 8

_f32 = mybir.dt.float32
_f32r = mybir.dt.float32r

_NC_CACHE = None


def _build_module():
    """One SPMD program: banded 2048^3 matmul + bias row, per core."""
    nc = bacc.Bacc("TRN2", target_bir_lowering=False, debug=False,
                   num_devices=N_CORES)

    lm_d = nc.dram_tensor("lmats", (3, P, P), _f32r, kind="ExternalInput").ap()
    bv_d = nc.dram_tensor("bv", (P, 1), _f32, kind="ExternalInput").ap()
    ws_d = nc.dram_tensor("ws", (N, N), _f32r, kind="ExternalInput").ap()
    bo_d = nc.dram_tensor("bout", (1, N), _f32, kind="ExternalInput").ap()
    y_d = nc.dram_tensor("y", (N, N), _f32, kind="ExternalOutput").ap()
    bn_d = nc.dram_tensor("bnew", (1, N), _f32, kind="ExternalOutput").ap()

    with tile.TileContext(nc) as tc:
        with (
            tc.tile_pool(name="const", bufs=1) as cpool,
            tc.tile_pool(name="wsp", bufs=3) as wpool,
            tc.tile_pool(name="yp", bufs=6) as ypool,
            tc.tile_pool(name="bnp", bufs=2) as bnpool,
            tc.tile_pool(name="acc", bufs=7, space="PSUM") as ppool,
            tc.tile_pool(name="bacc", bufs=1, space="PSUM") as pbpool,
        ):
            lsb = cpool.tile([P, 3, P], _f32r)
            for d in range(3):
                nc.sync.dma_start(out=lsb[:, d, :], in_=lm_d[d])
            bvsb = cpool.tile([P, 1], _f32)
            nc.sync.dma_start(out=bvsb[:], in_=bv_d[:])
            bosb = cpool.tile([1, N], _f32)
            nc.sync.dma_start(out=bosb[:], in_=bo_d[:])

            # software pipeline over k-chunks: in(kc) / compute(kc) / out(kc)
            # overlap via pool double/triple buffering.
            for kc in range(NCH):
                sl = slice(kc * KC, (kc + 1) * KC)
                ws_sb = []
                for ho in range(NH):
                    t = wpool.tile([P, KC], _f32r, tag=f"ws{ho}")
                    nc.sync.dma_start(out=t[:], in_=ws_d[ho * P:(ho + 1) * P, sl])
                    ws_sb.append(t)

                for h in range(NH):
                    acc = ppool.tile([P, KC], _f32, tag="acc")
                    dhs = [dh for dh in (-1, 0, 1) if 0 <= h - dh < NH]
                    for j, dh in enumerate(dhs):
                        ho = h - dh
                        nc.tensor.matmul(
                            acc[:],
                            lsb[:, dh + 1, :],
                            ws_sb[ho][:, :],
                            start=(j == 0),
                            stop=(j == len(dhs) - 1),
                        )
                    yt = ypool.tile([P, KC], _f32, tag="y")
                    nc.vector.tensor_copy(yt[:], acc[:])
                    nc.sync.dma_start(out=y_d[h * P:(h + 1) * P, sl], in_=yt[:])

                # bias row: T[k] = bvec . ws[:, k] -- accumulate the 16 tiles
                # on DVE, then a single M=1 matmul contracts the partitions.
                ba = bnpool.tile([P, KC], _f32, tag="ba")
                nc.vector.tensor_add(ba[:], ws_sb[0][:, :].bitcast(_f32),
                                     ws_sb[1][:, :].bitcast(_f32))
                for ho in range(2, NH):
                    nc.vector.tensor_add(ba[:], ba[:],
                                         ws_sb[ho][:, :].bitcast(_f32))
                pb = pbpool.tile([1, KC], _f32, tag="bacc")
                nc.tensor.matmul(pb[:], bvsb[:], ba[:], start=True, stop=True)
                bn = bnpool.tile([1, KC], _f32, tag="bn")
                nc.vector.tensor_add(bn[:], pb[:], bosb[:, sl])
                nc.sync.dma_start(out=bn_d[:, sl], in_=bn[:])

    nc.compile()
    return nc


def _get_nc():
    global _NC_CACHE
    if _NC_CACHE is None:
        _NC_CACHE = _build_module()
    return _NC_CACHE


def _stationaries(kernel: np.ndarray):
    """lhsT matrices: L[dh+1][(wo,co),(w,ci)] = K[dh+1, w-wo+1, ci, co]."""
    L = np.zeros((3, P, P), np.float32)
    k = np.asarray(kernel, np.float32)
    for dhi in range(3):
        for wo in range(W):
            for w in range(W):
                dw = w - wo
                if -1 <= dw <= 1:
                    L[dhi, 8 * wo:8 * wo + 8, 8 * w:8 * w + 8] = k[dhi, dw + 1].T
    return L


def run_device(inputs: dict, trace: bool = False):
    """Shard over 8 cores, run the bass kernel, gather. Returns (outs, results)."""
    nc = _get_nc()
    L = _stationaries(inputs["kernel"])
    bv = np.tile(np.asarray(inputs["bias"], np.float32), W).reshape(P, 1)

    in_maps = []
    for c in range(N_CORES):
        b, ul = c // 2, c % 2
        ws = inputs["w_out_u"] if ul == 0 else inputs["w_out_l"]
        bo = inputs["b_out_u"] if ul == 0 else inputs["b_out_l"]
        in_maps.append({
            "lmats": L,
            "bv": bv,
            "ws": np.ascontiguousarray(np.asarray(ws[b, 0], np.float32)),
            "bout": np.asarray(bo[b, 0], np.float32).reshape(1, N),
        })

    res = run_bass_kernel_spmd(nc, in_maps, core_ids=list(range(N_CORES)),
                               trace=trace)

    wu = np.stack([res.results[2 * b]["y"] for b in range(B)])[:, None]
    wl = np.stack([res.results[2 * b + 1]["y"] for b in range(B)])[:, None]
    bu = np.stack([res.results[2 * b]["bnew"][0] for b in range(B)])[:, None]
    bl = np.stack([res.results[2 * b + 1]["bnew"][0] for b in range(B)])[:, None]
    return (wu, bu, wl, bl), res


def kernel(**inputs) -> tuple:
    outs, _ = run_device(inputs, trace=False)
    return outs


# revision 8
# speedup vs baseline: 1.1151x; 1.1051x over previous
"""Trainium2 Bass kernel for nn_BackwardConv2D (CROWN linear-bound backward conv).

Math: the reference materializes the dense conv matrix Wm (n_dim x n_out) of a
3x3 SAME conv (H=W=16, CIN=COUT=8) and contracts it with w_out_u / w_out_l
per batch:  Y[b] = Wm @ Ws[b]  (the pos/neg split is linear and sums back to
the original), plus bias rows  T[b,k] = bvec . Ws[b,:,k].

Structure exploited here:
  * i = (h, w, ci) flat = 128*h + 8*w + ci ; o = (ho, wo, co) flat likewise.
    Wm[i, o] = K[h-ho+1, w-wo+1, ci, co], zero unless |h-ho|<=1 and |w-wo|<=1.
    => block-tridiagonal in 128-row tiles: output tile h only contracts with
       input tiles ho in {h-1, h, h+1}; and the 128x128 block depends only on
       dh = h-ho, so there are just 3 distinct stationary matrices.
  * 8 independent (batch, upper/lower) GEMMs -> one per NeuronCore.

Per core: banded matmul (46 nonzero 128x128x2048 tile-matmuls instead of 256
dense) + bias contraction as M=1 matmuls, all in float32r (FP22 1-pass).
"""

import numpy as np

import concourse.bass as bass  # noqa: F401  (env check)
import concourse.tile as tile
import concourse.mybir as mybir
from concourse import bacc
from concourse.bass_utils import run_bass_kernel_spmd

# Problem shape (hardcoded per harness contract)
B = 4
H = W = 16
CIN = COUT = 8
N = 2048          # n_dim = n_out = n_back = H*W*CIN
P = 128           # partition tile (one h-row of (w, ci))
NH = 16           # number of 128-row tiles (= H)
KC = 512          # k-chunk (one PSUM bank of fp32)
NCH = N // KC     # 4
N_CORES = 8

_f32 = mybir.dt.float32
_f32r = mybir.dt.float32r

_NC_CACHE = None


def _build_module():
    """One SPMD program: banded 2048^3 matmul + bias row, per core."""
    nc = bacc.Bacc("TRN2", target_bir_lowering=False, debug=False,
                   num_devices=N_CORES)

    lm_d = nc.dram_tensor("lmats", (3, P, P), _f32r, kind="ExternalInput").ap()
    bv_d = nc.dram_tensor("bv", (P, 1), _f32, kind="ExternalInput").ap()
    ws_d = nc.dram_tensor("ws", (N, N), _f32r, kind="ExternalInput").ap()
    bo_d = nc.dram_tensor("bout", (1, N), _f32, kind="ExternalInput").ap()
    y_d = nc.dram_tensor("y", (N, N), _f32, kind="ExternalOutput").ap()
    bn_d = nc.dram_tensor("bnew", (1, N), _f32, kind="ExternalOutput").ap()

    with tile.TileContext(nc) as tc:
        with (
            tc.tile_pool(name="const", bufs=1) as cpool,
            tc.tile_pool(name="wsp", bufs=3) as wpool,
            tc.tile_pool(name="yp", bufs=6) as ypool,
            tc.tile_pool(name="bnp", bufs=2) as bnpool,
            tc.tile_pool(name="acc", bufs=7, space="PSUM") as ppool,
            tc.tile_pool(name="bacc", bufs=1, space="PSUM") as pbpool,
        ):
            lsb = cpool.tile([P, 3, P], _f32r)
            for d in range(3):
                nc.sync.dma_start(out=lsb[:, d, :], in_=lm_d[d])
            bvsb = cpool.tile([P, 1], _f32)
            nc.sync.dma_start(out=bvsb[:], in_=bv_d[:])
            bosb = cpool.tile([1, N], _f32)
            nc.sync.dma_start(out=bosb[:], in_=bo_d[:])

            # software pipeline over k-chunks: in(kc) / compute(kc) / out(kc)
            # overlap via pool double/triple buffering.
            for kc in range(NCH):
                sl = slice(kc * KC, (kc + 1) * KC)
                ws_sb = []
                for ho in range(NH):
                    t = wpool.tile([P, KC], _f32r, tag=f"ws{ho}")
                    nc.gpsimd.dma_start(out=t[:], in_=ws_d[ho * P:(ho + 1) * P, sl])
                    ws_sb.append(t)

                for h in range(NH):
                    acc = ppool.tile([P, KC], _f32, tag="acc")
                    dhs = [dh for dh in (-1, 0, 1) if 0 <= h - dh < NH]
                    for j, dh in enumerate(dhs):
                        ho = h - dh
                        nc.tensor.matmul(
                            acc[:],
                            lsb[:, dh + 1, :],
                            ws_sb[ho][:, :],
                            start=(j == 0),
                            stop=(j == len(dhs) - 1),
                        )
                    yt = ypool.tile([P, KC], _f32, tag="y")
                    nc.any.tensor_copy(yt[:, :KC // 2], acc[:, :KC // 2])
                    nc.any.tensor_copy(yt[:, KC // 2:], acc[:, KC // 2:])
                    nc.sync.dma_start(out=y_d[h * P:(h + 1) * P, sl], in_=yt[:])

                # bias row: T[k] = bvec . ws[:, k] -- accumulate the 16 tiles
                # on DVE, then a single M=1 matmul contracts the partitions.
                ba = bnpool.tile([P, KC], _f32, tag="ba")
                nc.vector.tensor_add(ba[:], ws_sb[0][:, :].bitcast(_f32),
                                     ws_sb[1][:, :].bitcast(_f32))
                for ho in range(2, NH):
                    nc.vector.tensor_add(ba[:], ba[:],
                                         ws_sb[ho][:, :].bitcast(_f32))
                pb = pbpool.tile([1, KC], _f32, tag="bacc")
                nc.tensor.matmul(pb[:], bvsb[:], ba[:], start=True, stop=True)
                bn = bnpool.tile([1, KC], _f32, tag="bn")
                nc.vector.tensor_add(bn[:], pb[:], bosb[:, sl])
                nc.sync.dma_start(out=bn_d[:, sl], in_=bn[:])

    nc.compile()
    return nc


def _get_nc():
    global _NC_CACHE
    if _NC_CACHE is None:
        _NC_CACHE = _build_module()
    return _NC_CACHE


def _stationaries(kernel: np.ndarray):
    """lhsT matrices: L[dh+1][(wo,co),(w,ci)] = K[dh+1, w-wo+1, ci, co]."""
    L = np.zeros((3, P, P), np.float32)
    k = np.asarray(kernel, np.float32)
    for dhi in range(3):
        for wo in range(W):
            for w in range(W):
                dw = w - wo
                if -1 <= dw <= 1:
                    L[dhi, 8 * wo:8 * wo + 8, 8 * w:8 * w + 8] = k[dhi, dw + 1].T
    return L


def run_device(inputs: dict, trace: bool = False):
    """Shard over 8 cores, run the bass kernel, gather. Returns (outs, results)."""
    nc = _get_nc()
    L = _stationaries(inputs["kernel"])
    bv = np.tile(np.asarray(inputs["bias"], np.float32), W).reshape(P, 1)

    in_maps = []
    for c in range(N_CORES):
        b, ul = c // 2, c % 2
        ws = inputs["w_out_u"] if ul == 0 else inputs["w_out_l"]
        bo = inputs["b_out_u"] if ul == 0 else inputs["b_out_l"]
        in_maps.append({
            "lmats": L,
            "bv": bv,
            "ws": np.ascontiguousarray(np.asarray(ws[b, 0], np.float32)),
            "bout": np.asarray(bo[b, 0], np.float32).reshape(1, N),
        })

    res = run_bass_kernel_spmd(nc, in_maps, core_ids=list(range(N_CORES)),
                               trace=trace)

    wu = np.stack([res.results[2 * b]["y"] for b in range(B)])[:, None]
    wl = np.stack([res.results[2 * b + 1]["y"] for b in range(B)])[:, None]
    bu = np.stack([res.results[2 * b]["bnew"][0] for b in range(B)])[:, None]
    bl = np.stack([res.results[2 * b + 1]["bnew"][0] for b in range(B)])[:, None]
    return (wu, bu, wl, bl), res


def kernel(**inputs) -> tuple:
    outs, _ = run_device(inputs, trace=False)
    return outs


# revision 9
# speedup vs baseline: 1.1724x; 1.0514x over previous
"""Trainium2 Bass kernel for nn_BackwardConv2D (CROWN linear-bound backward conv).

Math: the reference materializes the dense conv matrix Wm (n_dim x n_out) of a
3x3 SAME conv (H=W=16, CIN=COUT=8) and contracts it with w_out_u / w_out_l
per batch:  Y[b] = Wm @ Ws[b]  (the pos/neg split is linear and sums back to
the original), plus bias rows  T[b,k] = bvec . Ws[b,:,k].

Structure exploited here:
  * i = (h, w, ci) flat = 128*h + 8*w + ci ; o = (ho, wo, co) flat likewise.
    Wm[i, o] = K[h-ho+1, w-wo+1, ci, co], zero unless |h-ho|<=1 and |w-wo|<=1.
    => block-tridiagonal in 128-row tiles: output tile h only contracts with
       input tiles ho in {h-1, h, h+1}; and the 128x128 block depends only on
       dh = h-ho, so there are just 3 distinct stationary matrices.
  * 8 independent (batch, upper/lower) GEMMs -> one per NeuronCore.

Per core: banded matmul (46 nonzero 128x128x2048 tile-matmuls instead of 256
dense) + bias contraction as M=1 matmuls, all in float32r (FP22 1-pass).
"""

import numpy as np

import concourse.bass as bass  # noqa: F401  (env check)
import concourse.tile as tile
import concourse.mybir as mybir
from concourse import bacc
from concourse.bass_utils import run_bass_kernel_spmd

# Problem shape (hardcoded per harness contract)
B = 4
H = W = 16
CIN = COUT = 8
N = 2048          # n_dim = n_out = n_back = H*W*CIN
P = 128           # partition tile (one h-row of (w, ci))
NH = 16           # number of 128-row tiles (= H)
KC = 512          # k-chunk (one PSUM bank of fp32)
NCH = N // KC     # 4
N_CORES = 8

_f32 = mybir.dt.float32
_f32r = mybir.dt.float32r

_NC_CACHE = None


def _build_module():
    """One SPMD program: banded 2048^3 matmul + bias row, per core."""
    nc = bacc.Bacc("TRN2", target_bir_lowering=False, debug=False,
                   num_devices=N_CORES)

    lm_d = nc.dram_tensor("lmats", (3, P, P), _f32r, kind="ExternalInput").ap()
    bv_d = nc.dram_tensor("bv", (P, 1), _f32, kind="ExternalInput").ap()
    ws_d = nc.dram_tensor("ws", (N, N), _f32r, kind="ExternalInput").ap()
    bo_d = nc.dram_tensor("bout", (1, N), _f32, kind="ExternalInput").ap()
    y_d = nc.dram_tensor("y", (N, N), _f32, kind="ExternalOutput").ap()
    bn_d = nc.dram_tensor("bnew", (1, N), _f32, kind="ExternalOutput").ap()

    with tile.TileContext(nc) as tc:
        with (
            tc.tile_pool(name="const", bufs=1) as cpool,
            tc.tile_pool(name="wsp", bufs=3) as wpool,
            tc.tile_pool(name="yp", bufs=6) as ypool,
            tc.tile_pool(name="bnp", bufs=2) as bnpool,
            tc.tile_pool(name="acc", bufs=7, space="PSUM") as ppool,
            tc.tile_pool(name="bacc", bufs=1, space="PSUM") as pbpool,
        ):
            lsb = cpool.tile([P, 3, P], _f32r)
            for d in range(3):
                nc.sync.dma_start(out=lsb[:, d, :], in_=lm_d[d])
            bvsb = cpool.tile([P, 1], _f32)
            nc.sync.dma_start(out=bvsb[:], in_=bv_d[:])
            bosb = cpool.tile([1, N], _f32)
            nc.sync.dma_start(out=bosb[:], in_=bo_d[:])

            # software pipeline over k-chunks: in(kc) / compute(kc) / out(kc)
            # overlap via pool double/triple buffering. Tiles are grouped 4
            # h-rows per DMA to amortize trigger cost (1MB per transfer).
            G = 4                 # h-tiles per DMA group
            NG = NH // G          # 4 groups
            for kc in range(NCH):
                sl = slice(kc * KC, (kc + 1) * KC)
                wg = []
                for g in range(NG):
                    t = wpool.tile([P, G, KC], _f32r, tag=f"wg{g}")
                    src = ws_d[g * G * P:(g + 1) * G * P, sl].rearrange(
                        "(a p) k -> p a k", p=P)
                    nc.gpsimd.dma_start(out=t[:], in_=src)
                    wg.append(t)

                def ws_tile(ho):
                    return wg[ho // G][:, ho % G, :]

                yg = None
                for h in range(NH):
                    if h % G == 0:
                        yg = ypool.tile([P, G, KC], _f32, tag="yg")
                    acc = ppool.tile([P, KC], _f32, tag="acc")
                    dhs = [dh for dh in (-1, 0, 1) if 0 <= h - dh < NH]
                    for j, dh in enumerate(dhs):
                        nc.tensor.matmul(
                            acc[:],
                            lsb[:, dh + 1, :],
                            ws_tile(h - dh),
                            start=(j == 0),
                            stop=(j == len(dhs) - 1),
                        )
                    i = h % G
                    nc.any.tensor_copy(yg[:, i, :KC // 2], acc[:, :KC // 2])
                    nc.any.tensor_copy(yg[:, i, KC // 2:], acc[:, KC // 2:])
                    if h % G == G - 1:
                        g = h // G
                        dst = y_d[g * G * P:(g + 1) * G * P, sl].rearrange(
                            "(a p) k -> p a k", p=P)
                        nc.sync.dma_start(out=dst, in_=yg[:])

                # bias row: T[k] = bvec . ws[:, k] -- accumulate the 16 tiles
                # on DVE, then a single M=1 matmul contracts the partitions.
                ba = bnpool.tile([P, KC], _f32, tag="ba")
                nc.vector.tensor_add(ba[:], ws_tile(0).bitcast(_f32),
                                     ws_tile(1).bitcast(_f32))
                for ho in range(2, NH):
                    nc.vector.tensor_add(ba[:], ba[:],
                                         ws_tile(ho).bitcast(_f32))
                pb = pbpool.tile([1, KC], _f32, tag="bacc")
                nc.tensor.matmul(pb[:], bvsb[:], ba[:], start=True, stop=True)
                bn = bnpool.tile([1, KC], _f32, tag="bn")
                nc.vector.tensor_add(bn[:], pb[:], bosb[:, sl])
                nc.sync.dma_start(out=bn_d[:, sl], in_=bn[:])

    nc.compile()
    return nc


def _get_nc():
    global _NC_CACHE
    if _NC_CACHE is None:
        _NC_CACHE = _build_module()
    return _NC_CACHE


def _stationaries(kernel: np.ndarray):
    """lhsT matrices: L[dh+1][(wo,co),(w,ci)] = K[dh+1, w-wo+1, ci, co]."""
    L = np.zeros((3, P, P), np.float32)
    k = np.asarray(kernel, np.float32)
    for dhi in range(3):
        for wo in range(W):
            for w in range(W):
                dw = w - wo
                if -1 <= dw <= 1:
                    L[dhi, 8 * wo:8 * wo + 8, 8 * w:8 * w + 8] = k[dhi, dw + 1].T
    return L


def run_device(inputs: dict, trace: bool = False):
    """Shard over 8 cores, run the bass kernel, gather. Returns (outs, results)."""
    nc = _get_nc()
    L = _stationaries(inputs["kernel"])
    bv = np.tile(np.asarray(inputs["bias"], np.float32), W).reshape(P, 1)

    in_maps = []
    for c in range(N_CORES):
        b, ul = c // 2, c % 2
        ws = inputs["w_out_u"] if ul == 0 else inputs["w_out_l"]
        bo = inputs["b_out_u"] if ul == 0 else inputs["b_out_l"]
        in_maps.append({
            "lmats": L,
            "bv": bv,
            "ws": np.ascontiguousarray(np.asarray(ws[b, 0], np.float32)),
            "bout": np.asarray(bo[b, 0], np.float32).reshape(1, N),
        })

    res = run_bass_kernel_spmd(nc, in_maps, core_ids=list(range(N_CORES)),
                               trace=trace)

    wu = np.stack([res.results[2 * b]["y"] for b in range(B)])[:, None]
    wl = np.stack([res.results[2 * b + 1]["y"] for b in range(B)])[:, None]
    bu = np.stack([res.results[2 * b]["bnew"][0] for b in range(B)])[:, None]
    bl = np.stack([res.results[2 * b + 1]["bnew"][0] for b in range(B)])[:, None]
    return (wu, bu, wl, bl), res


def kernel(**inputs) -> tuple:
    outs, _ = run_device(inputs, trace=False)
    return outs


# revision 11
# speedup vs baseline: 1.1965x; 1.0206x over previous
"""Trainium2 Bass kernel for nn_BackwardConv2D (CROWN linear-bound backward conv).

Math: the reference materializes the dense conv matrix Wm (n_dim x n_out) of a
3x3 SAME conv (H=W=16, CIN=COUT=8) and contracts it with w_out_u / w_out_l
per batch:  Y[b] = Wm @ Ws[b]  (the pos/neg split is linear and sums back to
the original), plus bias rows  T[b,k] = bvec . Ws[b,:,k].

Structure exploited here:
  * i = (h, w, ci) flat = 128*h + 8*w + ci ; o = (ho, wo, co) flat likewise.
    Wm[i, o] = K[h-ho+1, w-wo+1, ci, co], zero unless |h-ho|<=1 and |w-wo|<=1.
    => block-tridiagonal in 128-row tiles: output tile h only contracts with
       input tiles ho in {h-1, h, h+1}; and the 128x128 block depends only on
       dh = h-ho, so there are just 3 distinct stationary matrices.
  * 8 independent (batch, upper/lower) GEMMs -> one per NeuronCore.

Per core: banded matmul (46 nonzero 128x128x2048 tile-matmuls instead of 256
dense) + bias contraction as M=1 matmuls, all in float32r (FP22 1-pass).
"""

import numpy as np

import concourse.bass as bass  # noqa: F401  (env check)
import concourse.tile as tile
import concourse.mybir as mybir
from concourse import bacc
from concourse.bass_utils import run_bass_kernel_spmd

# Problem shape (hardcoded per harness contract)
B = 4
H = W = 16
CIN = COUT = 8
N = 2048          # n_dim = n_out = n_back = H*W*CIN
P = 128           # partition tile (one h-row of (w, ci))
NH = 16           # number of 128-row tiles (= H)
KC = 512          # k-chunk (one PSUM bank of fp32)
NCH = N // KC     # 4
N_CORES = 8

_f32 = mybir.dt.float32
_f32r = mybir.dt.float32r

_NC_CACHE = None


def _build_module():
    """One SPMD program: banded 2048^3 matmul + bias row, per core."""
    nc = bacc.Bacc("TRN2", target_bir_lowering=False, debug=False,
                   num_devices=N_CORES)

    lm_d = nc.dram_tensor("lmats", (3, P, P), _f32r, kind="ExternalInput").ap()
    bv_d = nc.dram_tensor("bv", (P, 1), _f32, kind="ExternalInput").ap()
    ws_d = nc.dram_tensor("ws", (N, N), _f32r, kind="ExternalInput").ap()
    bo_d = nc.dram_tensor("bout", (1, N), _f32, kind="ExternalInput").ap()
    y_d = nc.dram_tensor("y", (N, N), _f32, kind="ExternalOutput").ap()
    bn_d = nc.dram_tensor("bnew", (1, N), _f32, kind="ExternalOutput").ap()

    with tile.TileContext(nc) as tc:
        with (
            tc.tile_pool(name="const", bufs=1) as cpool,
            tc.tile_pool(name="wsp", bufs=3) as wpool,
            tc.tile_pool(name="yp", bufs=6) as ypool,
            tc.tile_pool(name="bnp", bufs=2) as bnpool,
            tc.tile_pool(name="acc", bufs=7, space="PSUM") as ppool,
            tc.tile_pool(name="bacc", bufs=1, space="PSUM") as pbpool,
        ):
            lsb = cpool.tile([P, 3, P], _f32r)
            for d in range(3):
                nc.sync.dma_start(out=lsb[:, d, :], in_=lm_d[d])
            bvsb = cpool.tile([P, 1], _f32)
            nc.sync.dma_start(out=bvsb[:], in_=bv_d[:])
            bosb = cpool.tile([1, N], _f32)
            nc.sync.dma_start(out=bosb[:], in_=bo_d[:])

            # software pipeline over k-chunks: in(kc) / compute(kc) / out(kc)
            # overlap via pool double/triple buffering. Tiles are grouped 4
            # h-rows per DMA to amortize trigger cost (1MB per transfer).
            G = 4                 # h-tiles per DMA group
            NG = NH // G          # 4 groups
            for kc in range(NCH):
                sl = slice(kc * KC, (kc + 1) * KC)
                wg = []
                for g in range(NG):
                    t = wpool.tile([P, G, KC], _f32r, tag=f"wg{g}")
                    src = ws_d[g * G * P:(g + 1) * G * P, sl].rearrange(
                        "(a p) k -> p a k", p=P)
                    nc.gpsimd.dma_start(out=t[:], in_=src)
                    wg.append(t)

                def ws_tile(ho):
                    return wg[ho // G][:, ho % G, :]

                # bias row: T[k] = bvec . ws[:, k] -- per-group partial sums
                # on DVE (issued as each group lands), then combine + one M=1
                # matmul to contract the partitions.
                gsum = []
                yg = None
                for h in range(NH):
                    if h % G == 0:
                        yg = ypool.tile([P, G, KC], _f32, tag="yg")
                    acc = ppool.tile([P, KC], _f32, tag="acc")
                    dhs = [dh for dh in (-1, 0, 1) if 0 <= h - dh < NH]
                    for j, dh in enumerate(dhs):
                        nc.tensor.matmul(
                            acc[:],
                            lsb[:, dh + 1, :],
                            ws_tile(h - dh),
                            start=(j == 0),
                            stop=(j == len(dhs) - 1),
                        )
                    i = h % G
                    nc.any.tensor_copy(yg[:, i, :KC // 2], acc[:, :KC // 2])
                    nc.any.tensor_copy(yg[:, i, KC // 2:], acc[:, KC // 2:])
                    if h % G == G - 1:
                        g = h // G
                        dst = y_d[g * G * P:(g + 1) * G * P, sl].rearrange(
                            "(a p) k -> p a k", p=P)
                        nc.sync.dma_start(out=dst, in_=yg[:])
                        gs = bnpool.tile([P, KC], _f32, tag=f"gs{g}")
                        nc.vector.tensor_add(gs[:], wg[g][:, 0, :].bitcast(_f32),
                                             wg[g][:, 1, :].bitcast(_f32))
                        nc.vector.tensor_add(gs[:], gs[:],
                                             wg[g][:, 2, :].bitcast(_f32))
                        nc.vector.tensor_add(gs[:], gs[:],
                                             wg[g][:, 3, :].bitcast(_f32))
                        gsum.append(gs)
                ba = bnpool.tile([P, KC], _f32, tag="ba")
                nc.vector.tensor_add(ba[:], gsum[0][:], gsum[1][:])
                nc.vector.tensor_add(ba[:], ba[:], gsum[2][:])
                nc.vector.tensor_add(ba[:], ba[:], gsum[3][:])
                pb = pbpool.tile([1, KC], _f32, tag="bacc")
                nc.tensor.matmul(pb[:], bvsb[:], ba[:], start=True, stop=True)
                bn = bnpool.tile([1, KC], _f32, tag="bn")
                nc.vector.tensor_add(bn[:], pb[:], bosb[:, sl])
                nc.sync.dma_start(out=bn_d[:, sl], in_=bn[:])

    nc.compile()
    return nc


def _get_nc():
    global _NC_CACHE
    if _NC_CACHE is None:
        _NC_CACHE = _build_module()
    return _NC_CACHE


def _stationaries(kernel: np.ndarray):
    """lhsT matrices: L[dh+1][(wo,co),(w,ci)] = K[dh+1, w-wo+1, ci, co]."""
    L = np.zeros((3, P, P), np.float32)
    k = np.asarray(kernel, np.float32)
    for dhi in range(3):
        for wo in range(W):
            for w in range(W):
                dw = w - wo
                if -1 <= dw <= 1:
                    L[dhi, 8 * wo:8 * wo + 8, 8 * w:8 * w + 8] = k[dhi, dw + 1].T
    return L


def run_device(inputs: dict, trace: bool = False):
    """Shard over 8 cores, run the bass kernel, gather. Returns (outs, results)."""
    nc = _get_nc()
    L = _stationaries(inputs["kernel"])
    bv = np.tile(np.asarray(inputs["bias"], np.float32), W).reshape(P, 1)

    in_maps = []
    for c in range(N_CORES):
        b, ul = c // 2, c % 2
        ws = inputs["w_out_u"] if ul == 0 else inputs["w_out_l"]
        bo = inputs["b_out_u"] if ul == 0 else inputs["b_out_l"]
        in_maps.append({
            "lmats": L,
            "bv": bv,
            "ws": np.ascontiguousarray(np.asarray(ws[b, 0], np.float32)),
            "bout": np.asarray(bo[b, 0], np.float32).reshape(1, N),
        })

    res = run_bass_kernel_spmd(nc, in_maps, core_ids=list(range(N_CORES)),
                               trace=trace)

    wu = np.stack([res.results[2 * b]["y"] for b in range(B)])[:, None]
    wl = np.stack([res.results[2 * b + 1]["y"] for b in range(B)])[:, None]
    bu = np.stack([res.results[2 * b]["bnew"][0] for b in range(B)])[:, None]
    bl = np.stack([res.results[2 * b + 1]["bnew"][0] for b in range(B)])[:, None]
    return (wu, bu, wl, bl), res


def kernel(**inputs) -> tuple:
    outs, _ = run_device(inputs, trace=False)
    return outs


# revision 12
# speedup vs baseline: 1.4529x; 1.2143x over previous
"""Trainium2 Bass kernel for nn_BackwardConv2D (CROWN linear-bound backward conv).

Math: the reference materializes the dense conv matrix Wm (n_dim x n_out) of a
3x3 SAME conv (H=W=16, CIN=COUT=8) and contracts it with w_out_u / w_out_l
per batch:  Y[b] = Wm @ Ws[b]  (the pos/neg split is linear and sums back to
the original), plus bias rows  T[b,k] = bvec . Ws[b,:,k].

Structure exploited here:
  * i = (h, w, ci) flat = 128*h + 8*w + ci ; o = (ho, wo, co) flat likewise.
    Wm[i, o] = K[h-ho+1, w-wo+1, ci, co], zero unless |h-ho|<=1 and |w-wo|<=1.
    => block-tridiagonal in 128-row tiles: output tile h only contracts with
       input tiles ho in {h-1, h, h+1}; and the 128x128 block depends only on
       dh = h-ho, so there are just 3 distinct stationary matrices.
  * 8 independent (batch, upper/lower) GEMMs -> one per NeuronCore.

Per core: banded matmul (46 nonzero 128x128x2048 tile-matmuls instead of 256
dense) + bias contraction as M=1 matmuls, all in float32r (FP22 1-pass).
"""

import numpy as np

import concourse.bass as bass  # noqa: F401  (env check)
import concourse.tile as tile
import concourse.mybir as mybir
from concourse import bacc
from concourse.bass_utils import run_bass_kernel_spmd

# Problem shape (hardcoded per harness contract)
B = 4
H = W = 16
CIN = COUT = 8
N = 2048          # n_dim = n_out = n_back = H*W*CIN
P = 128           # partition tile (one h-row of (w, ci))
NH = 16           # number of 128-row tiles (= H)
KC = 512          # k-chunk (one PSUM bank of fp32)
NCH = N // KC     # 4
N_CORES = 8

_f32 = mybir.dt.float32
_f32r = mybir.dt.float32r
_f16 = mybir.dt.float16

# fp16 input path: halves the ws HBM traffic; PE upconverts fp16->FP22
# exactly, so only the host-side cast (2^-11 rounding) adds error.
USE_FP16 = True

_NC_CACHE = None


def _build_module():
    """One SPMD program: banded 2048^3 matmul + bias row, per core."""
    nc = bacc.Bacc("TRN2", target_bir_lowering=False, debug=False,
                   num_devices=N_CORES)

    _wdt = _f16 if USE_FP16 else _f32r
    lm_d = nc.dram_tensor("lmats", (3, P, P), _wdt, kind="ExternalInput").ap()
    bv_d = nc.dram_tensor("bv", (P, 1), _f32, kind="ExternalInput").ap()
    ws_d = nc.dram_tensor("ws", (N, N), _wdt, kind="ExternalInput").ap()
    bo_d = nc.dram_tensor("bout", (1, N), _f32, kind="ExternalInput").ap()
    y_d = nc.dram_tensor("y", (N, N), _f32, kind="ExternalOutput").ap()
    bn_d = nc.dram_tensor("bnew", (1, N), _f32, kind="ExternalOutput").ap()

    with tile.TileContext(nc) as tc:
        with (
            tc.tile_pool(name="const", bufs=1) as cpool,
            tc.tile_pool(name="wsp", bufs=3) as wpool,
            tc.tile_pool(name="yp", bufs=6) as ypool,
            tc.tile_pool(name="bnp", bufs=2) as bnpool,
            tc.tile_pool(name="acc", bufs=7, space="PSUM") as ppool,
            tc.tile_pool(name="bacc", bufs=1, space="PSUM") as pbpool,
        ):
            lsb = cpool.tile([P, 3, P], _wdt)
            for d in range(3):
                nc.sync.dma_start(out=lsb[:, d, :], in_=lm_d[d])
            bvsb = cpool.tile([P, 1], _f32)
            nc.sync.dma_start(out=bvsb[:], in_=bv_d[:])
            bosb = cpool.tile([1, N], _f32)
            nc.sync.dma_start(out=bosb[:], in_=bo_d[:])

            # software pipeline over k-chunks: in(kc) / compute(kc) / out(kc)
            # overlap via pool double/triple buffering. Tiles are grouped 4
            # h-rows per DMA to amortize trigger cost (1MB per transfer).
            G = 4                 # h-tiles per DMA group
            NG = NH // G          # 4 groups
            for kc in range(NCH):
                sl = slice(kc * KC, (kc + 1) * KC)
                wg = []
                for g in range(NG):
                    t = wpool.tile([P, G, KC], _wdt, tag=f"wg{g}")
                    src = ws_d[g * G * P:(g + 1) * G * P, sl].rearrange(
                        "(a p) k -> p a k", p=P)
                    nc.gpsimd.dma_start(out=t[:], in_=src)
                    wg.append(t)

                def ws_tile(ho):
                    return wg[ho // G][:, ho % G, :]

                # bias row: T[k] = bvec . ws[:, k] -- per-group partial sums
                # on DVE (issued as each group lands), then combine + one M=1
                # matmul to contract the partitions.
                gsum = []
                yg = None
                for h in range(NH):
                    if h % G == 0:
                        yg = ypool.tile([P, G, KC], _f32, tag="yg")
                    acc = ppool.tile([P, KC], _f32, tag="acc")
                    dhs = [dh for dh in (-1, 0, 1) if 0 <= h - dh < NH]
                    for j, dh in enumerate(dhs):
                        nc.tensor.matmul(
                            acc[:],
                            lsb[:, dh + 1, :],
                            ws_tile(h - dh),
                            start=(j == 0),
                            stop=(j == len(dhs) - 1),
                        )
                    i = h % G
                    nc.any.tensor_copy(yg[:, i, :KC // 2], acc[:, :KC // 2])
                    nc.any.tensor_copy(yg[:, i, KC // 2:], acc[:, KC // 2:])
                    if h % G == G - 1:
                        g = h // G
                        dst = y_d[g * G * P:(g + 1) * G * P, sl].rearrange(
                            "(a p) k -> p a k", p=P)
                        nc.sync.dma_start(out=dst, in_=yg[:])
                        def _dve_view(ap):
                            return ap if USE_FP16 else ap.bitcast(_f32)
                        gs = bnpool.tile([P, KC], _f32, tag=f"gs{g}")
                        nc.vector.tensor_add(gs[:], _dve_view(wg[g][:, 0, :]),
                                             _dve_view(wg[g][:, 1, :]))
                        nc.vector.tensor_add(gs[:], gs[:],
                                             _dve_view(wg[g][:, 2, :]))
                        nc.vector.tensor_add(gs[:], gs[:],
                                             _dve_view(wg[g][:, 3, :]))
                        gsum.append(gs)
                ba = bnpool.tile([P, KC], _f32, tag="ba")
                nc.vector.tensor_add(ba[:], gsum[0][:], gsum[1][:])
                nc.vector.tensor_add(ba[:], ba[:], gsum[2][:])
                nc.vector.tensor_add(ba[:], ba[:], gsum[3][:])
                pb = pbpool.tile([1, KC], _f32, tag="bacc")
                nc.tensor.matmul(pb[:], bvsb[:], ba[:], start=True, stop=True)
                bn = bnpool.tile([1, KC], _f32, tag="bn")
                nc.vector.tensor_add(bn[:], pb[:], bosb[:, sl])
                nc.sync.dma_start(out=bn_d[:, sl], in_=bn[:])

    nc.compile()
    return nc


def _get_nc():
    global _NC_CACHE
    if _NC_CACHE is None:
        _NC_CACHE = _build_module()
    return _NC_CACHE


def _stationaries(kernel: np.ndarray):
    """lhsT matrices: L[dh+1][(wo,co),(w,ci)] = K[dh+1, w-wo+1, ci, co]."""
    L = np.zeros((3, P, P), np.float32)
    k = np.asarray(kernel, np.float32)
    for dhi in range(3):
        for wo in range(W):
            for w in range(W):
                dw = w - wo
                if -1 <= dw <= 1:
                    L[dhi, 8 * wo:8 * wo + 8, 8 * w:8 * w + 8] = k[dhi, dw + 1].T
    return L


def run_device(inputs: dict, trace: bool = False):
    """Shard over 8 cores, run the bass kernel, gather. Returns (outs, results)."""
    nc = _get_nc()
    wdt = np.float16 if USE_FP16 else np.float32
    L = _stationaries(inputs["kernel"]).astype(wdt)
    bv = np.tile(np.asarray(inputs["bias"], np.float32), W).reshape(P, 1)

    in_maps = []
    for c in range(N_CORES):
        b, ul = c // 2, c % 2
        ws = inputs["w_out_u"] if ul == 0 else inputs["w_out_l"]
        bo = inputs["b_out_u"] if ul == 0 else inputs["b_out_l"]
        in_maps.append({
            "lmats": L,
            "bv": bv,
            "ws": np.ascontiguousarray(np.asarray(ws[b, 0], wdt)),
            "bout": np.asarray(bo[b, 0], np.float32).reshape(1, N),
        })

    res = run_bass_kernel_spmd(nc, in_maps, core_ids=list(range(N_CORES)),
                               trace=trace)

    wu = np.stack([res.results[2 * b]["y"] for b in range(B)])[:, None]
    wl = np.stack([res.results[2 * b + 1]["y"] for b in range(B)])[:, None]
    bu = np.stack([res.results[2 * b]["bnew"][0] for b in range(B)])[:, None]
    bl = np.stack([res.results[2 * b + 1]["bnew"][0] for b in range(B)])[:, None]
    return (wu, bu, wl, bl), res


def kernel(**inputs) -> tuple:
    outs, _ = run_device(inputs, trace=False)
    return outs


# revision 13
# speedup vs baseline: 1.8644x; 1.2832x over previous
"""Trainium2 Bass kernel for nn_BackwardConv2D (CROWN linear-bound backward conv).

Math: the reference materializes the dense conv matrix Wm (n_dim x n_out) of a
3x3 SAME conv (H=W=16, CIN=COUT=8) and contracts it with w_out_u / w_out_l
per batch:  Y[b] = Wm @ Ws[b]  (the pos/neg split is linear and sums back to
the original), plus bias rows  T[b,k] = bvec . Ws[b,:,k].

Structure exploited here:
  * i = (h, w, ci) flat = 128*h + 8*w + ci ; o = (ho, wo, co) flat likewise.
    Wm[i, o] = K[h-ho+1, w-wo+1, ci, co], zero unless |h-ho|<=1 and |w-wo|<=1.
    => block-tridiagonal in 128-row tiles: output tile h only contracts with
       input tiles ho in {h-1, h, h+1}; and the 128x128 block depends only on
       dh = h-ho, so there are just 3 distinct stationary matrices.
  * 8 independent (batch, upper/lower) GEMMs -> one per NeuronCore.

Per core: banded matmul (46 nonzero 128x128x2048 tile-matmuls instead of 256
dense) + bias contraction as M=1 matmuls, all in float32r (FP22 1-pass).
"""

import numpy as np

import concourse.bass as bass  # noqa: F401  (env check)
import concourse.tile as tile
import concourse.mybir as mybir
from concourse import bacc
from concourse.bass_utils import run_bass_kernel_spmd

# Problem shape (hardcoded per harness contract)
B = 4
H = W = 16
CIN = COUT = 8
N = 2048          # n_dim = n_out = n_back = H*W*CIN
P = 128           # partition tile (one h-row of (w, ci))
NH = 16           # number of 128-row tiles (= H)
KC = 512          # k-chunk (one PSUM bank of fp32)
NCH = N // KC     # 4
N_CORES = 8

_f32 = mybir.dt.float32
_f32r = mybir.dt.float32r
_f16 = mybir.dt.float16

# fp16 input path: halves the ws HBM traffic; PE upconverts fp16->FP22
# exactly, so only the host-side cast (2^-11 rounding) adds error.
USE_FP16 = True
USE_FP16_OUT = True

_NC_CACHE = None


def _build_module():
    """One SPMD program: banded 2048^3 matmul + bias row, per core."""
    nc = bacc.Bacc("TRN2", target_bir_lowering=False, debug=False,
                   num_devices=N_CORES)

    _wdt = _f16 if USE_FP16 else _f32r
    lm_d = nc.dram_tensor("lmats", (3, P, P), _wdt, kind="ExternalInput").ap()
    bv_d = nc.dram_tensor("bv", (P, 1), _f32, kind="ExternalInput").ap()
    ws_d = nc.dram_tensor("ws", (N, N), _wdt, kind="ExternalInput").ap()
    bo_d = nc.dram_tensor("bout", (1, N), _f32, kind="ExternalInput").ap()
    _ydt = _f16 if USE_FP16_OUT else _f32
    y_d = nc.dram_tensor("y", (N, N), _ydt, kind="ExternalOutput").ap()
    bn_d = nc.dram_tensor("bnew", (1, N), _f32, kind="ExternalOutput").ap()

    with tile.TileContext(nc) as tc:
        with (
            tc.tile_pool(name="const", bufs=1) as cpool,
            tc.tile_pool(name="wsp", bufs=3) as wpool,
            tc.tile_pool(name="yp", bufs=6) as ypool,
            tc.tile_pool(name="bnp", bufs=2) as bnpool,
            tc.tile_pool(name="acc", bufs=7, space="PSUM") as ppool,
            tc.tile_pool(name="bacc", bufs=1, space="PSUM") as pbpool,
        ):
            lsb = cpool.tile([P, 3, P], _wdt)
            for d in range(3):
                nc.sync.dma_start(out=lsb[:, d, :], in_=lm_d[d])
            bvsb = cpool.tile([P, 1], _f32)
            nc.sync.dma_start(out=bvsb[:], in_=bv_d[:])
            bosb = cpool.tile([1, N], _f32)
            nc.sync.dma_start(out=bosb[:], in_=bo_d[:])

            # software pipeline over k-chunks: in(kc) / compute(kc) / out(kc)
            # overlap via pool double/triple buffering. Tiles are grouped 4
            # h-rows per DMA to amortize trigger cost (1MB per transfer).
            G = 4                 # h-tiles per DMA group
            NG = NH // G          # 4 groups
            for kc in range(NCH):
                sl = slice(kc * KC, (kc + 1) * KC)
                wg = []
                for g in range(NG):
                    t = wpool.tile([P, G, KC], _wdt, tag=f"wg{g}")
                    src = ws_d[g * G * P:(g + 1) * G * P, sl].rearrange(
                        "(a p) k -> p a k", p=P)
                    nc.gpsimd.dma_start(out=t[:], in_=src)
                    wg.append(t)

                def ws_tile(ho):
                    return wg[ho // G][:, ho % G, :]

                # bias row: T[k] = bvec . ws[:, k] -- per-group partial sums
                # on DVE (issued as each group lands), then combine + one M=1
                # matmul to contract the partitions.
                gsum = []
                yg = None
                for h in range(NH):
                    if h % G == 0:
                        yg = ypool.tile([P, G, KC], _ydt, tag="yg")
                    acc = ppool.tile([P, KC], _f32, tag="acc")
                    dhs = [dh for dh in (-1, 0, 1) if 0 <= h - dh < NH]
                    for j, dh in enumerate(dhs):
                        nc.tensor.matmul(
                            acc[:],
                            lsb[:, dh + 1, :],
                            ws_tile(h - dh),
                            start=(j == 0),
                            stop=(j == len(dhs) - 1),
                        )
                    i = h % G
                    nc.any.tensor_copy(yg[:, i, :KC // 2], acc[:, :KC // 2])
                    nc.any.tensor_copy(yg[:, i, KC // 2:], acc[:, KC // 2:])
                    if h % G == G - 1:
                        g = h // G
                        dst = y_d[g * G * P:(g + 1) * G * P, sl].rearrange(
                            "(a p) k -> p a k", p=P)
                        nc.sync.dma_start(out=dst, in_=yg[:])
                        def _dve_view(ap):
                            return ap if USE_FP16 else ap.bitcast(_f32)
                        gs = bnpool.tile([P, KC], _f32, tag=f"gs{g}")
                        nc.vector.tensor_add(gs[:], _dve_view(wg[g][:, 0, :]),
                                             _dve_view(wg[g][:, 1, :]))
                        nc.vector.tensor_add(gs[:], gs[:],
                                             _dve_view(wg[g][:, 2, :]))
                        nc.vector.tensor_add(gs[:], gs[:],
                                             _dve_view(wg[g][:, 3, :]))
                        gsum.append(gs)
                ba = bnpool.tile([P, KC], _f32, tag="ba")
                nc.vector.tensor_add(ba[:], gsum[0][:], gsum[1][:])
                nc.vector.tensor_add(ba[:], ba[:], gsum[2][:])
                nc.vector.tensor_add(ba[:], ba[:], gsum[3][:])
                pb = pbpool.tile([1, KC], _f32, tag="bacc")
                nc.tensor.matmul(pb[:], bvsb[:], ba[:], start=True, stop=True)
                bn = bnpool.tile([1, KC], _f32, tag="bn")
                nc.vector.tensor_add(bn[:], pb[:], bosb[:, sl])
                nc.sync.dma_start(out=bn_d[:, sl], in_=bn[:])

    nc.compile()
    return nc


def _get_nc():
    global _NC_CACHE
    if _NC_CACHE is None:
        _NC_CACHE = _build_module()
    return _NC_CACHE


def _stationaries(kernel: np.ndarray):
    """lhsT matrices: L[dh+1][(wo,co),(w,ci)] = K[dh+1, w-wo+1, ci, co]."""
    L = np.zeros((3, P, P), np.float32)
    k = np.asarray(kernel, np.float32)
    for dhi in range(3):
        for wo in range(W):
            for w in range(W):
                dw = w - wo
                if -1 <= dw <= 1:
                    L[dhi, 8 * wo:8 * wo + 8, 8 * w:8 * w + 8] = k[dhi, dw + 1].T
    return L


def run_device(inputs: dict, trace: bool = False):
    """Shard over 8 cores, run the bass kernel, gather. Returns (outs, results)."""
    nc = _get_nc()
    wdt = np.float16 if USE_FP16 else np.float32
    L = _stationaries(inputs["kernel"]).astype(wdt)
    bv = np.tile(np.asarray(inputs["bias"], np.float32), W).reshape(P, 1)

    in_maps = []
    for c in range(N_CORES):
        b, ul = c // 2, c % 2
        ws = inputs["w_out_u"] if ul == 0 else inputs["w_out_l"]
        bo = inputs["b_out_u"] if ul == 0 else inputs["b_out_l"]
        in_maps.append({
            "lmats": L,
            "bv": bv,
            "ws": np.ascontiguousarray(np.asarray(ws[b, 0], wdt)),
            "bout": np.asarray(bo[b, 0], np.float32).reshape(1, N),
        })

    res = run_bass_kernel_spmd(nc, in_maps, core_ids=list(range(N_CORES)),
                               trace=trace)

    wu = np.stack([res.results[2 * b]["y"].astype(np.float32) for b in range(B)])[:, None]
    wl = np.stack([res.results[2 * b + 1]["y"].astype(np.float32) for b in range(B)])[:, None]
    bu = np.stack([res.results[2 * b]["bnew"][0] for b in range(B)])[:, None]
    bl = np.stack([res.results[2 * b + 1]["bnew"][0] for b in range(B)])[:, None]
    return (wu, bu, wl, bl), res


def kernel(**inputs) -> tuple:
    outs, _ = run_device(inputs, trace=False)
    return outs


# revision 14
# speedup vs baseline: 1.9381x; 1.0395x over previous
"""Trainium2 Bass kernel for nn_BackwardConv2D (CROWN linear-bound backward conv).

Math: the reference materializes the dense conv matrix Wm (n_dim x n_out) of a
3x3 SAME conv (H=W=16, CIN=COUT=8) and contracts it with w_out_u / w_out_l
per batch:  Y[b] = Wm @ Ws[b]  (the pos/neg split is linear and sums back to
the original), plus bias rows  T[b,k] = bvec . Ws[b,:,k].

Structure exploited here:
  * i = (h, w, ci) flat = 128*h + 8*w + ci ; o = (ho, wo, co) flat likewise.
    Wm[i, o] = K[h-ho+1, w-wo+1, ci, co], zero unless |h-ho|<=1 and |w-wo|<=1.
    => block-tridiagonal in 128-row tiles: output tile h only contracts with
       input tiles ho in {h-1, h, h+1}; and the 128x128 block depends only on
       dh = h-ho, so there are just 3 distinct stationary matrices.
  * 8 independent (batch, upper/lower) GEMMs -> one per NeuronCore.

Per core: banded matmul (46 nonzero 128x128x2048 tile-matmuls instead of 256
dense) + bias contraction as M=1 matmuls, all in float32r (FP22 1-pass).
"""

import numpy as np

import concourse.bass as bass  # noqa: F401  (env check)
import concourse.tile as tile
import concourse.mybir as mybir
from concourse import bacc
from concourse.bass_utils import run_bass_kernel_spmd

# Problem shape (hardcoded per harness contract)
B = 4
H = W = 16
CIN = COUT = 8
N = 2048          # n_dim = n_out = n_back = H*W*CIN
P = 128           # partition tile (one h-row of (w, ci))
NH = 16           # number of 128-row tiles (= H)
KC = 512          # k-chunk (one PSUM bank of fp32)
NCH = N // KC     # 4
N_CORES = 8

_f32 = mybir.dt.float32
_f32r = mybir.dt.float32r
_f16 = mybir.dt.float16

# fp16 input path: halves the ws HBM traffic; PE upconverts fp16->FP22
# exactly, so only the host-side cast (2^-11 rounding) adds error.
USE_FP16 = True
USE_FP16_OUT = True

_NC_CACHE = None


def _build_module():
    """One SPMD program: banded 2048^3 matmul + bias row, per core."""
    nc = bacc.Bacc("TRN2", target_bir_lowering=False, debug=False,
                   num_devices=N_CORES)

    _wdt = _f16 if USE_FP16 else _f32r
    lm_d = nc.dram_tensor("lmats", (3, P, P), _wdt, kind="ExternalInput").ap()
    bv_d = nc.dram_tensor("bv", (P, 1), _f32, kind="ExternalInput").ap()
    ws_d = nc.dram_tensor("ws", (N, N), _wdt, kind="ExternalInput").ap()
    bo_d = nc.dram_tensor("bout", (1, N), _f32, kind="ExternalInput").ap()
    _ydt = _f16 if USE_FP16_OUT else _f32
    y_d = nc.dram_tensor("y", (N, N), _ydt, kind="ExternalOutput").ap()
    bn_d = nc.dram_tensor("bnew", (1, N), _f32, kind="ExternalOutput").ap()

    with tile.TileContext(nc) as tc:
        with (
            tc.tile_pool(name="const", bufs=1) as cpool,
            tc.tile_pool(name="wsp", bufs=4) as wpool,
            tc.tile_pool(name="yp", bufs=8) as ypool,
            tc.tile_pool(name="bnp", bufs=2) as bnpool,
            tc.tile_pool(name="acc", bufs=7, space="PSUM") as ppool,
            tc.tile_pool(name="bacc", bufs=1, space="PSUM") as pbpool,
        ):
            lsb = cpool.tile([P, 3, P], _wdt)
            for d in range(3):
                nc.sync.dma_start(out=lsb[:, d, :], in_=lm_d[d])
            bvsb = cpool.tile([P, 1], _f32)
            nc.sync.dma_start(out=bvsb[:], in_=bv_d[:])
            bosb = cpool.tile([1, N], _f32)
            nc.sync.dma_start(out=bosb[:], in_=bo_d[:])

            # software pipeline over k-chunks: in(kc) / compute(kc) / out(kc)
            # overlap via pool double/triple buffering. Tiles are grouped 4
            # h-rows per DMA to amortize trigger cost (1MB per transfer).
            G = 4                 # h-tiles per DMA group
            NG = NH // G          # 4 groups
            for kc in range(NCH):
                sl = slice(kc * KC, (kc + 1) * KC)
                wg = []
                for g in range(NG):
                    t = wpool.tile([P, G, KC], _wdt, tag=f"wg{g}")
                    src = ws_d[g * G * P:(g + 1) * G * P, sl].rearrange(
                        "(a p) k -> p a k", p=P)
                    nc.gpsimd.dma_start(out=t[:], in_=src)
                    wg.append(t)

                def ws_tile(ho):
                    return wg[ho // G][:, ho % G, :]

                # bias row: T[k] = bvec . ws[:, k] -- per-group partial sums
                # on DVE (issued as each group lands), then combine + one M=1
                # matmul to contract the partitions.
                gsum = []
                yg = None
                for h in range(NH):
                    if h % G == 0:
                        yg = ypool.tile([P, G, KC], _ydt, tag="yg")
                    acc = ppool.tile([P, KC], _f32, tag="acc")
                    dhs = [dh for dh in (-1, 0, 1) if 0 <= h - dh < NH]
                    for j, dh in enumerate(dhs):
                        nc.tensor.matmul(
                            acc[:],
                            lsb[:, dh + 1, :],
                            ws_tile(h - dh),
                            start=(j == 0),
                            stop=(j == len(dhs) - 1),
                        )
                    i = h % G
                    nc.any.tensor_copy(yg[:, i, :], acc[:])
                    if h % G == G - 1:
                        g = h // G
                        dst = y_d[g * G * P:(g + 1) * G * P, sl].rearrange(
                            "(a p) k -> p a k", p=P)
                        nc.sync.dma_start(out=dst, in_=yg[:])
                        def _dve_view(ap):
                            return ap if USE_FP16 else ap.bitcast(_f32)
                        gs = bnpool.tile([P, KC], _f32, tag=f"gs{g}")
                        nc.vector.tensor_add(gs[:], _dve_view(wg[g][:, 0, :]),
                                             _dve_view(wg[g][:, 1, :]))
                        nc.vector.tensor_add(gs[:], gs[:],
                                             _dve_view(wg[g][:, 2, :]))
                        nc.vector.tensor_add(gs[:], gs[:],
                                             _dve_view(wg[g][:, 3, :]))
                        gsum.append(gs)
                ba = bnpool.tile([P, KC], _f32, tag="ba")
                nc.vector.tensor_add(ba[:], gsum[0][:], gsum[1][:])
                nc.vector.tensor_add(ba[:], ba[:], gsum[2][:])
                nc.vector.tensor_add(ba[:], ba[:], gsum[3][:])
                pb = pbpool.tile([1, KC], _f32, tag="bacc")
                nc.tensor.matmul(pb[:], bvsb[:], ba[:], start=True, stop=True)
                bn = bnpool.tile([1, KC], _f32, tag="bn")
                nc.vector.tensor_add(bn[:], pb[:], bosb[:, sl])
                nc.sync.dma_start(out=bn_d[:, sl], in_=bn[:])

    nc.compile()
    return nc


def _get_nc():
    global _NC_CACHE
    if _NC_CACHE is None:
        _NC_CACHE = _build_module()
    return _NC_CACHE


def _stationaries(kernel: np.ndarray):
    """lhsT matrices: L[dh+1][(wo,co),(w,ci)] = K[dh+1, w-wo+1, ci, co]."""
    L = np.zeros((3, P, P), np.float32)
    k = np.asarray(kernel, np.float32)
    for dhi in range(3):
        for wo in range(W):
            for w in range(W):
                dw = w - wo
                if -1 <= dw <= 1:
                    L[dhi, 8 * wo:8 * wo + 8, 8 * w:8 * w + 8] = k[dhi, dw + 1].T
    return L


def run_device(inputs: dict, trace: bool = False):
    """Shard over 8 cores, run the bass kernel, gather. Returns (outs, results)."""
    nc = _get_nc()
    wdt = np.float16 if USE_FP16 else np.float32
    L = _stationaries(inputs["kernel"]).astype(wdt)
    bv = np.tile(np.asarray(inputs["bias"], np.float32), W).reshape(P, 1)

    in_maps = []
    for c in range(N_CORES):
        b, ul = c // 2, c % 2
        ws = inputs["w_out_u"] if ul == 0 else inputs["w_out_l"]
        bo = inputs["b_out_u"] if ul == 0 else inputs["b_out_l"]
        in_maps.append({
            "lmats": L,
            "bv": bv,
            "ws": np.ascontiguousarray(np.asarray(ws[b, 0], wdt)),
            "bout": np.asarray(bo[b, 0], np.float32).reshape(1, N),
        })

    res = run_bass_kernel_spmd(nc, in_maps, core_ids=list(range(N_CORES)),
                               trace=trace)

    wu = np.stack([res.results[2 * b]["y"].astype(np.float32) for b in range(B)])[:, None]
    wl = np.stack([res.results[2 * b + 1]["y"].astype(np.float32) for b in range(B)])[:, None]
    bu = np.stack([res.results[2 * b]["bnew"][0] for b in range(B)])[:, None]
    bl = np.stack([res.results[2 * b + 1]["bnew"][0] for b in range(B)])[:, None]
    return (wu, bu, wl, bl), res


def kernel(**inputs) -> tuple:
    outs, _ = run_device(inputs, trace=False)
    return outs


# revision 19
# speedup vs baseline: 2.0023x; 1.0332x over previous
"""Trainium2 Bass kernel for nn_BackwardConv2D (CROWN linear-bound backward conv).

Math: the reference materializes the dense conv matrix Wm (n_dim x n_out) of a
3x3 SAME conv (H=W=16, CIN=COUT=8) and contracts it with w_out_u / w_out_l
per batch:  Y[b] = Wm @ Ws[b]  (the pos/neg split is linear and sums back to
the original), plus bias rows  T[b,k] = bvec . Ws[b,:,k].

Structure exploited here:
  * i = (h, w, ci) flat = 128*h + 8*w + ci ; o = (ho, wo, co) flat likewise.
    Wm[i, o] = K[h-ho+1, w-wo+1, ci, co], zero unless |h-ho|<=1 and |w-wo|<=1.
    => block-tridiagonal in 128-row tiles: output tile h only contracts with
       input tiles ho in {h-1, h, h+1}; and the 128x128 block depends only on
       dh = h-ho, so there are just 3 distinct stationary matrices.
  * 8 independent (batch, upper/lower) GEMMs -> one per NeuronCore.

Per core: banded matmul (46 nonzero 128x128-contraction tile-matmuls per
512-wide k-chunk instead of 256 dense) with fp16 operands (PE upconverts
fp16 -> FP22 exactly; only the host cast rounds), fp16 output staging,
bias contraction via DVE partial sums + one M=1 matmul. Everything is
software-pipelined over 4 k-chunks: in(kc+1) | compute(kc) | out(kc).
Measured: ~66 us HW exec, rel err ~3.6e-4 (DMA ~47 us floor + PE ~47 us,
preamble/epilogue ~14 us framework overhead).
"""

import numpy as np

import concourse.bass as bass  # noqa: F401  (env check)
import concourse.tile as tile
import concourse.mybir as mybir
from concourse import bacc
from concourse.bass_utils import run_bass_kernel_spmd

# Problem shape (hardcoded per harness contract)
B = 4
H = W = 16
CIN = COUT = 8
N = 2048          # n_dim = n_out = n_back = H*W*CIN
P = 128           # partition tile (one h-row of (w, ci))
NH = 16           # number of 128-row tiles (= H)
KC = 512          # k-chunk (one PSUM bank of fp32)
NCH = N // KC     # 4
N_CORES = 8

_f32 = mybir.dt.float32
_f32r = mybir.dt.float32r
_f16 = mybir.dt.float16

# fp16 input path: halves the ws HBM traffic; PE upconverts fp16->FP22
# exactly, so only the host-side cast (2^-11 rounding) adds error.
USE_FP16 = True
USE_FP16_OUT = True

_NC_CACHE = None


def _build_module():
    """One SPMD program: banded 2048^3 matmul + bias row, per core."""
    nc = bacc.Bacc("TRN2", target_bir_lowering=False, debug=False,
                   num_devices=N_CORES)

    _wdt = _f16 if USE_FP16 else _f32r
    lm_d = nc.dram_tensor("lmats", (3, P, P), _wdt, kind="ExternalInput").ap()
    bv_d = nc.dram_tensor("bv", (P, 1), _f32, kind="ExternalInput").ap()
    ws_d = nc.dram_tensor("ws", (N, N), _wdt, kind="ExternalInput").ap()
    bo_d = nc.dram_tensor("bout", (1, N), _f32, kind="ExternalInput").ap()
    _ydt = _f16 if USE_FP16_OUT else _f32
    y_d = nc.dram_tensor("y", (N, N), _ydt, kind="ExternalOutput").ap()
    bn_d = nc.dram_tensor("bnew", (1, N), _f32, kind="ExternalOutput").ap()

    with tile.TileContext(nc) as tc:
        with (
            tc.tile_pool(name="const", bufs=1) as cpool,
            tc.tile_pool(name="wsp", bufs=4) as wpool,
            tc.tile_pool(name="yp", bufs=8) as ypool,
            tc.tile_pool(name="bnp", bufs=2) as bnpool,
            tc.tile_pool(name="acc", bufs=7, space="PSUM") as ppool,
            tc.tile_pool(name="bacc", bufs=1, space="PSUM") as pbpool,
        ):
            lsb = cpool.tile([P, 3, P], _wdt)
            for d in range(3):
                nc.sync.dma_start(out=lsb[:, d, :], in_=lm_d[d])
            bvsb = cpool.tile([P, 1], _f32)
            nc.sync.dma_start(out=bvsb[:], in_=bv_d[:])
            bosb = cpool.tile([1, N], _f32)
            nc.sync.dma_start(out=bosb[:], in_=bo_d[:])

            # software pipeline over k-chunks: in(kc) / compute(kc) / out(kc)
            # overlap via pool double/triple buffering. Tiles are grouped 4
            # h-rows per DMA to amortize trigger cost (1MB per transfer).
            G = 4                 # h-tiles per DMA group
            NG = NH // G          # 4 groups
            for kc in range(NCH):
                sl = slice(kc * KC, (kc + 1) * KC)
                wg = []
                for g in range(NG):
                    t = wpool.tile([P, G, KC], _wdt, tag=f"wg{g}")
                    src = ws_d[g * G * P:(g + 1) * G * P, sl].rearrange(
                        "(a p) k -> p a k", p=P)
                    nc.gpsimd.dma_start(out=t[:], in_=src)
                    wg.append(t)

                def ws_tile(ho):
                    return wg[ho // G][:, ho % G, :]

                # bias row: T[k] = bvec . ws[:, k] -- per-group partial sums
                # on DVE (issued as each group lands), then combine + one M=1
                # matmul to contract the partitions.
                gsum = []
                yg = None
                for h in range(NH):
                    if h % G == 0:
                        yg = ypool.tile([P, G, KC], _ydt, tag="yg")
                    acc = ppool.tile([P, KC], _f32, tag="acc")
                    dhs = [dh for dh in (-1, 0, 1) if 0 <= h - dh < NH]
                    for j, dh in enumerate(dhs):
                        nc.tensor.matmul(
                            acc[:],
                            lsb[:, dh + 1, :],
                            ws_tile(h - dh),
                            start=(j == 0),
                            stop=(j == len(dhs) - 1),
                        )
                    i = h % G
                    nc.any.tensor_copy(yg[:, i, :], acc[:])
                    if h % G == G - 1:
                        g = h // G
                        dst = y_d[g * G * P:(g + 1) * G * P, sl].rearrange(
                            "(a p) k -> p a k", p=P)
                        nc.sync.dma_start(out=dst, in_=yg[:])
                        def _dve_view(ap):
                            return ap if USE_FP16 else ap.bitcast(_f32)
                        gs = bnpool.tile([P, KC], _f32, tag=f"gs{g}")
                        nc.vector.tensor_add(gs[:], _dve_view(wg[g][:, 0, :]),
                                             _dve_view(wg[g][:, 1, :]))
                        nc.vector.tensor_add(gs[:], gs[:],
                                             _dve_view(wg[g][:, 2, :]))
                        nc.vector.tensor_add(gs[:], gs[:],
                                             _dve_view(wg[g][:, 3, :]))
                        gsum.append(gs)
                ba = bnpool.tile([P, KC], _f32, tag="ba")
                nc.vector.tensor_add(ba[:], gsum[0][:], gsum[1][:])
                nc.vector.tensor_add(ba[:], ba[:], gsum[2][:])
                nc.vector.tensor_add(ba[:], ba[:], gsum[3][:])
                pb = pbpool.tile([1, KC], _f32, tag="bacc")
                nc.tensor.matmul(pb[:], bvsb[:], ba[:], start=True, stop=True)
                bn = bnpool.tile([1, KC], _f32, tag="bn")
                nc.vector.tensor_add(bn[:], pb[:], bosb[:, sl])
                nc.sync.dma_start(out=bn_d[:, sl], in_=bn[:])

    nc.compile()
    return nc


def _get_nc():
    global _NC_CACHE
    if _NC_CACHE is None:
        _NC_CACHE = _build_module()
    return _NC_CACHE


def _stationaries(kernel: np.ndarray):
    """lhsT matrices: L[dh+1][(wo,co),(w,ci)] = K[dh+1, w-wo+1, ci, co]."""
    L = np.zeros((3, P, P), np.float32)
    k = np.asarray(kernel, np.float32)
    for dhi in range(3):
        for wo in range(W):
            for w in range(W):
                dw = w - wo
                if -1 <= dw <= 1:
                    L[dhi, 8 * wo:8 * wo + 8, 8 * w:8 * w + 8] = k[dhi, dw + 1].T
    return L


_RUNNER = None


def _get_runner():
    """Persistent jitted SPMD executor (run_bass_via_pjrt re-jits per call;
    this caches the jax.jit so repeat kernel() calls skip XLA recompile)."""
    global _RUNNER
    if _RUNNER is not None:
        return _RUNNER
    import jax
    import concourse.mybir as _mb
    from concourse import bass2jax
    from jax.sharding import Mesh, PartitionSpec
    from jax.experimental.shard_map import shard_map

    nc = _get_nc()
    bass2jax.install_neuronx_cc_hook()
    partition_name = (nc.partition_id_tensor.name
                      if nc.partition_id_tensor else None)
    in_names, out_names, out_avals, zero_outs = [], [], [], []
    for alloc in nc.m.functions[0].allocations:
        if not isinstance(alloc, _mb.MemoryLocationSet):
            continue
        name = alloc.memorylocations[0].name
        if alloc.kind == "ExternalInput":
            if name != partition_name:
                in_names.append(name)
        elif alloc.kind == "ExternalOutput":
            shape = tuple(alloc.tensor_shape)
            dtype = _mb.dt.np(alloc.dtype)
            out_names.append(name)
            out_avals.append(jax.core.ShapedArray(shape, dtype))
            zero_outs.append(np.zeros(shape, dtype))
    n_params, n_outs = len(in_names), len(out_avals)
    all_names = in_names + out_names
    if partition_name is not None:
        all_names = all_names + [partition_name]

    def _body(*args):
        operands = list(args)
        if partition_name is not None:
            operands.append(bass2jax.partition_id_tensor())
        outs = bass2jax._bass_exec_p.bind(
            *operands,
            out_avals=tuple(out_avals),
            in_names=tuple(all_names),
            out_names=tuple(out_names),
            lowering_input_output_aliases=(),
            sim_require_finite=True,
            sim_require_nnan=True,
            nc=nc,
        )
        return tuple(outs)

    devices = jax.devices()[:N_CORES]
    mesh = Mesh(np.asarray(devices), ("core",))
    in_specs = (PartitionSpec("core"),) * (n_params + n_outs)
    out_specs = (PartitionSpec("core"),) * n_outs
    sharded = jax.jit(
        shard_map(_body, mesh=mesh, in_specs=in_specs, out_specs=out_specs,
                  check_rep=False),
        donate_argnums=tuple(range(n_params, n_params + n_outs)),
        keep_unused=True,
    )

    def run(in_maps):
        concat_in = [
            np.concatenate([np.asarray(in_maps[c][nm]) for c in range(N_CORES)],
                           axis=0)
            for nm in in_names
        ]
        concat_zeros = [
            np.zeros((N_CORES * z.shape[0], *z.shape[1:]), z.dtype)
            for z in zero_outs
        ]
        out_arrs = sharded(*concat_in, *concat_zeros)
        return [
            {nm: np.asarray(out_arrs[i]).reshape(N_CORES, *out_avals[i].shape)[c]
             for i, nm in enumerate(out_names)}
            for c in range(N_CORES)
        ]

    _RUNNER = run
    return _RUNNER


def _make_in_maps(inputs: dict):
    wdt = np.float16 if USE_FP16 else np.float32
    L = _stationaries(inputs["kernel"]).astype(wdt)
    bv = np.tile(np.asarray(inputs["bias"], np.float32), W).reshape(P, 1)

    in_maps = []
    for c in range(N_CORES):
        b, ul = c // 2, c % 2
        ws = inputs["w_out_u"] if ul == 0 else inputs["w_out_l"]
        bo = inputs["b_out_u"] if ul == 0 else inputs["b_out_l"]
        in_maps.append({
            "lmats": L,
            "bv": bv,
            "ws": np.ascontiguousarray(np.asarray(ws[b, 0], wdt)),
            "bout": np.asarray(bo[b, 0], np.float32).reshape(1, N),
        })
    return in_maps


def _gather(results):
    wu = np.stack([results[2 * b]["y"].astype(np.float32)
                   for b in range(B)])[:, None]
    wl = np.stack([results[2 * b + 1]["y"].astype(np.float32)
                   for b in range(B)])[:, None]
    bu = np.stack([results[2 * b]["bnew"][0] for b in range(B)])[:, None]
    bl = np.stack([results[2 * b + 1]["bnew"][0] for b in range(B)])[:, None]
    return (wu, bu, wl, bl)


def run_device(inputs: dict, trace: bool = False):
    """Shard over 8 cores, run the bass kernel, gather. Returns (outs, res).

    trace=True goes through run_bass_kernel_spmd for NTFF profiling (res has
    exec_time_ns); trace=False uses the cached jitted runner (res is None).
    """
    in_maps = _make_in_maps(inputs)
    if trace:
        nc = _get_nc()
        res = run_bass_kernel_spmd(nc, in_maps, core_ids=list(range(N_CORES)),
                                   trace=True)
        return _gather(res.results), res
    results = _get_runner()(in_maps)
    return _gather(results), None


def kernel(**inputs) -> tuple:
    outs, _ = run_device(inputs, trace=False)
    return outs
